# revision 71
# baseline (speedup 1.0000x reference)
"""Trainium2 Bass kernel: fused Linear + InstanceNorm + (normed + y) * y.

Math:
    h = x @ w.T + b                      # [B, OUT]
    mean/var per row over OUT features
    normed = (h - mean) * rsqrt(var+eps) * nw + nb
    out = (normed + y) * y

Key restructuring (all exact algebra, no approximation):
  * mean subtraction folds into the weights:  h - mean(h) = x @ (w - wbar)^T + (b - bbar)
    where wbar[i] = mean_o w[o,i], bbar = mean(b).  The main matmul uses
    centered, norm_w-scaled weights  w'' = (w - wbar) * nw  and produces
    g = (h - mean) * nw - b''  with b'' = (b - bbar) * nw  a per-column constant.
  * per-row stats come from tiny extra matmul columns (x-side identities):
        mean   = x.wbar + bbar
        E[h^2] = x^T M x + 2 x.mb + msq,   M = w^T w/OUT, mb = w^T b/OUT, msq = mean(b^2)
    so no reduction over the 2048-wide feature dim is ever needed.  eps is
    folded into msq so that  e2b = E[h^2] + eps  and  std = sqrt(e2b - mu^2).
  * the per-column constants (b'', nb) are applied as a rank-2 PSUM-accumulate
    matmul:  PSUM += ones (x) b'' + std (x) nb,  then one ScalarE pass
    multiplies by invstd per row:  t = invstd * PSUM = (h-mean)*invstd*nw + nb.
  * final: u = t + y, o = u * y  (two VectorE passes per half-tile).

Memory-bound problem, so everything streamed is bf16 (rel-err budget 2e-2,
measured ~7e-3): y loads are bf16 (halves the dominant HBM read), x is packed
bf16 (transposed + natural per 128-row tile), weights bf16, and the whole
elementwise tail runs in bf16 so VectorE tensor_tensor hits its 2x perf mode.
The output store is issued from GpSimd (SWDGE) with an inline bf16->f32 cast,
which also keeps store waits out of the Sync engine's load-issue stream.

Scheduling notes: walrus allows only ONE semaphore wait on a Matmult/Ldweights
(S3_LW struct).  The emission order below is arranged so that, under Tile's
per-engine vector clock, every PE instruction needs at most one wait:
  - stats for tile i+1 are issued one tile ahead, so the PE transpose of tile i
    waits only on ACT's sqrt (both so2 columns are written by ACT only);
  - every SBUF tile feeding a matmul weight load has a single producer engine.

Data-parallel over the batch dim across 8 NeuronCores; every core runs the
same program on its 4096-row shard.
"""

import os

import numpy as np
import ml_dtypes

B, IN, OUT = 32768, 128, 2048
N_CORES = 8
P = 128
EPS = 1e-5

STATS_N = 132        # wbar | mb | M (128 cols) | 2 pad
HALF = OUT // 2      # psum half-tile width (2 banks)

_CACHE = {}

LAST_RESULT = None


def _build_nc(n_rows):
    import concourse.bass as bass
    import concourse.tile as tile
    from concourse import bacc, mybir
    from concourse.bass import ts
    from concourse.masks import make_identity

    # "gp"   = o in bf16, gpsimd SWDGE store with inline bf16->f32 cast
    # "gpns" = o in f32 (DVE 1x mul), gpsimd SWDGE store, no cast
    # "f32"  = o in f32 (DVE 1x mul), sync HWDGE store
    store_mode = os.environ.get("KSTORE", "gp")

    f32 = mybir.dt.float32
    bf16 = mybir.dt.bfloat16
    AF = mybir.ActivationFunctionType
    ALU = mybir.AluOpType

    nc = bacc.Bacc()
    # one DRAM row of xp = one partition line of a 256-row pair-tile:
    # [A.T row | B.T row] for the even/odd 128-row compute tiles
    xp_d = nc.dram_tensor("xp", [n_rows // 2, 2 * P], bf16, kind="ExternalInput")
    y_d = nc.dram_tensor("y", [n_rows, OUT], bf16, kind="ExternalInput")
    wm_d = nc.dram_tensor("wt_main", [P, OUT], bf16, kind="ExternalInput")
    ws_d = nc.dram_tensor("wt_stats", [P, STATS_N], bf16, kind="ExternalInput")
    rk_d = nc.dram_tensor("rank_rhs", [P, OUT], bf16, kind="ExternalInput")
    c_d = nc.dram_tensor("consts", [P, 2], f32, kind="ExternalInput")
    out_d = nc.dram_tensor("out", [n_rows, OUT], f32, kind="ExternalOutput")

    T = n_rows // P

    with tile.TileContext(nc) as tc:
        with (
            tc.tile_pool(name="singles", bufs=1) as singles,
            tc.tile_pool(name="xin", bufs=6) as xin,
            tc.tile_pool(name="yin", bufs=6) as yin,
            tc.tile_pool(name="stats", bufs=4) as stats,
            tc.tile_pool(name="mid", bufs=4) as mid,
            tc.tile_pool(name="osb", bufs=3) as osb,
            tc.tile_pool(name="pm", bufs=3, space="PSUM") as pm,
            tc.tile_pool(name="pstat", bufs=2, space="PSUM") as pstat,
        ):
            # ---- constants ----
            wm_sb = singles.tile([P, OUT], bf16)
            nc.sync.dma_start(wm_sb[:], wm_d[:])
            ws_sb = singles.tile([P, STATS_N], bf16)
            nc.sync.dma_start(ws_sb[:], ws_d[:])
            rk_sb = singles.tile([P, OUT], bf16)
            nc.sync.dma_start(rk_sb[:], rk_d[:])
            ident = singles.tile([P, P], f32)
            make_identity(nc, ident[:])
            epst = singles.tile([P, 1], f32)
            nc.vector.memset(epst[:], EPS)
            consts_sb = singles.tile([P, 2], f32)
            nc.sync.dma_start(consts_sb[:], c_d[:])
            bbar_t = consts_sb[:, 0:1]
            msq_t = consts_sb[:, 1:2]   # msq + eps

            # ---- pipelined per-tile state ----
            xT_tiles = {}
            y_tiles = {}
            ps_s_tiles = {}
            chain_out = {}  # t -> so2
            inv_out = {}
            sigma_out = {}
            tile_state = {}

            def emit_dma(g):
                """Load a 256-row pair-block g (compute tiles 2g, 2g+1).

                The natural row-major reshape puts DRAM rows (2p, 2p+1) on
                partition p: 8 KiB contiguous per partition for y — big DMA
                descriptors.  Tile A = even rows (cols 0:OUT), B = odd rows."""
                if 2 * g >= T:
                    return
                xp_t = xin.tile([P, 2 * P], bf16, tag="xp")
                nc.sync.dma_start(xp_t[:], xp_d[g * P : (g + 1) * P, :])
                y_t = yin.tile([P, 2 * OUT], bf16)
                nc.sync.dma_start(y_t[:], y_d[2 * g * P : 2 * (g + 1) * P, :])
                xT_tiles[2 * g] = xp_t[:, 0:P]
                xT_tiles[2 * g + 1] = xp_t[:, P : 2 * P]
                y_tiles[2 * g] = y_t[:, 0:OUT]
                y_tiles[2 * g + 1] = y_t[:, OUT : 2 * OUT]

            def emit_stats_mm(t):
                if t >= T:
                    return
                # one PSUM bank per buf: stats cols 0:132, transpose scratch
                # at cols 384:512 (written by PE only after all stats reads)
                ps_s = pstat.tile([P, 512], f32, tag="ps")
                nc.tensor.matmul(
                    ps_s[:, 0:STATS_N], xT_tiles[t], ws_sb[:], start=True, stop=True
                )
                ps_s_tiles[t] = ps_s

            chain_mid_state = {}

            def emit_chain(t):
                """Early stats chain for tile t: everything whose deps are
                ready as soon as ps_s lands (keeps ACT free for sigma)."""
                if t >= T:
                    return
                ps_s = ps_s_tiles[t]
                # E[h^2] via Cholesky: cols 2:130 hold z = x L with M = L L^T,
                # so sum(z*z) = x^T M x
                scratch = stats.tile([P, P], f32, tag="scratch")
                nc.scalar.activation(scratch[:], ps_s[:, 2:130], AF.Square)
                e2 = stats.tile([P, 1], f32, tag="e2")
                nc.vector.tensor_reduce(
                    e2[:], scratch[:], axis=mybir.AxisListType.X, op=ALU.add
                )
                t2 = stats.tile([P, 1], f32, tag="t2")
                nc.vector.tensor_scalar(
                    t2[:], ps_s[:, 1:2], 2.0, msq_t[:], ALU.mult, ALU.add
                )
                mu = stats.tile([P, 1], f32, tag="mu")
                nc.vector.tensor_scalar(mu[:], ps_s[:, 0:1], bbar_t[:], None, ALU.add)
                chain_mid_state[t] = (e2, t2, mu)

            def emit_chain_mid(t):
                """Late stats chain: emitted after this iteration's sigma ops
                so the ACT sqrt never head-of-line-blocks sigma."""
                if t >= T:
                    return
                e2, t2, mu = chain_mid_state.pop(t)
                mu2 = stats.tile([P, 1], f32, tag="mu2")
                nc.gpsimd.tensor_mul(mu2[:], mu[:], mu[:])
                e2b = stats.tile([P, 1], f32, tag="e2b")
                nc.gpsimd.tensor_add(e2b[:], e2[:], t2[:])
                var = stats.tile([P, 1], f32, tag="var")
                nc.gpsimd.tensor_tensor(var[:], e2b[:], mu2[:], ALU.subtract)
                # so2 = [ones | std]; the ones column is identical every tile,
                # so it is written only while the 4 pool buffers first rotate
                so2 = stats.tile([P, 2], f32, tag="so2")
                if t < 4 or os.environ.get("KONES") == "all":
                    nc.scalar.activation(
                        so2[:, 0:1], epst[:], AF.Copy, bias=1.0, scale=0.0
                    )
                nc.scalar.activation(so2[:, 1:2], var[:], AF.Sqrt)
                chain_out[t] = so2

            def emit_chain_b(t):
                if t >= T:
                    return
                so2 = chain_out[t]
                inv = stats.tile([P, 1], f32, tag="inv")
                nc.vector.reciprocal(inv[:], so2[:, 1:2])
                inv_out[t] = inv

            def emit_transpose_lhst2(t):
                if t >= T:
                    return
                so2 = chain_out.pop(t)
                inv = inv_out.pop(t)
                ps_t = ps_s_tiles.pop(t)[0:2, 384:512]
                nc.tensor.transpose(ps_t, so2[:], ident[:])
                lhsT2 = stats.tile([2, P], bf16, tag="lhsT2")
                nc.scalar.copy(lhsT2[:], ps_t)
                tile_state[t] = (lhsT2, inv)

            def emit_mains_ranks_sigma(t):
                xT_t = xT_tiles.pop(t)
                lhsT2, inv = tile_state.pop(t)
                ps0 = pm.tile([P, HALF], f32, tag="pm")
                ps1 = pm.tile([P, HALF], f32, tag="pm")
                ps_h = [ps0, ps1]
                # all 4 main matmuls share the xT stationary; then all 4
                # rank-2 matmuls share lhsT2 — 2 weight swaps per tile, not 8
                for h in (0, 1):
                    for j in range(HALF // 512):
                        c0 = h * HALF + j * 512
                        nc.tensor.matmul(
                            ps_h[h][:, ts(j, 512)],
                            xT_t,
                            wm_sb[:, c0 : c0 + 512],
                            start=True,
                            stop=False,
                        )
                ts_list = []
                for h in (0, 1):
                    for j in range(HALF // 512):
                        c0 = h * HALF + j * 512
                        nc.tensor.matmul(
                            ps_h[h][:, ts(j, 512)],
                            lhsT2[:],
                            rk_sb[0:2, c0 : c0 + 512],
                            start=False,
                            stop=True,
                        )
                    # sigma for this half right after its rank-stop, so the
                    # PSUM buffer frees before the next tile's mains need it
                    t_h = mid.tile([P, HALF], bf16, tag="t")
                    nc.scalar.mul(t_h[:], ps_h[h][:], inv[:])
                    ts_list.append(t_h)
                sigma_out[t] = ts_list

            o_pair = {}

            def emit_y(t):
                if t < 0:
                    return
                ts_list = sigma_out.pop(t)
                y_t = y_tiles.pop(t)
                g, sub = divmod(t, 2)
                if sub == 0:
                    o256 = osb.tile([P, 2 * OUT], bf16 if store_mode == "gp" else f32)
                    o_pair[g] = o256
                else:
                    o256 = o_pair[g]
                for h, t_h in ((0, ts_list[0]), (1, ts_list[1])):
                    ys = y_t[:, h * HALF : (h + 1) * HALF]
                    u = mid.tile([P, HALF], bf16, tag="u")
                    nc.vector.tensor_add(u[:], t_h[:], ys)
                    c0 = sub * OUT + h * HALF
                    nc.vector.tensor_mul(o256[:, c0 : c0 + HALF], u[:], ys)
                if sub == 1 or t == T - 1:
                    o_sl = o256[:, 0 : (sub + 1) * OUT]
                    dma_eng = nc.sync if store_mode == "f32" else nc.gpsimd
                    dma_eng.dma_start(
                        out_d[2 * g * P : (2 * g + sub + 1) * P, :], o_sl
                    )
                    o_pair.pop(g)

            # ---- preamble pipeline fill ----
            for g0 in range(4):
                emit_dma(g0)
            emit_stats_mm(0)
            emit_chain(0)
            emit_chain_mid(0)
            emit_chain_b(0)
            emit_transpose_lhst2(0)
            emit_stats_mm(1)
            emit_chain(1)
            emit_chain_mid(1)
            emit_chain_b(1)
            emit_transpose_lhst2(1)

            # ---- steady-state: the whole stats chain runs two tiles ahead
            # so each ps_s PSUM buffer is consumed within its own iteration ----
            for t in range(T):
                emit_stats_mm(t + 2)
                emit_chain(t + 2)
                emit_mains_ranks_sigma(t)
                emit_chain_mid(t + 2)
                emit_y(t - 1)
                emit_chain_b(t + 2)
                emit_transpose_lhst2(t + 2)
                if t % 2 == 0:
                    emit_dma(t // 2 + 4)
            emit_y(T - 1)

    nc.finalize()
    return nc


def check_wait_budget(nc):
    """Every Matmult/Ldweights must carry at most one semaphore wait."""
    bad = []
    j = nc.to_json()
    for f in j["functions"]:
        for blk in f["blocks"]:
            for ins in blk["instructions"]:
                if ins.get("type") in ("Matmult", "Ldweights"):
                    waits = (ins.get("sync") or {}).get("on_wait") or []
                    if len(waits) > 1:
                        bad.append((ins.get("name"), ins.get("type"), len(waits)))
    return bad


def _host_prep(x, y, linear_w, linear_b, norm_w, norm_b):
    """Host-side derived tensors (f64 precompute, cast bf16/f32)."""
    bf = ml_dtypes.bfloat16
    w64 = linear_w.astype(np.float64)
    b64 = linear_b.astype(np.float64)
    nw64 = norm_w.astype(np.float64)
    nb64 = norm_b.astype(np.float64)

    wbar = w64.mean(axis=0)            # [IN]
    bbar = b64.mean()
    M = (w64.T @ w64) / OUT            # [IN, IN]
    L = np.linalg.cholesky(M)          # M = L @ L.T, so x^T M x = |x L|^2
    mb = (w64.T @ b64) / OUT           # [IN]
    msq = float((b64 * b64).mean())
    wpp = (w64 - wbar[None, :]) * nw64[:, None]   # [OUT, IN]
    bpp = (b64 - bbar) * nw64                     # [OUT]

    wt_main = np.ascontiguousarray(wpp.T.astype(bf))
    wt_stats = np.zeros((P, STATS_N), bf)
    wt_stats[:, 0] = wbar.astype(bf)
    wt_stats[:, 1] = mb.astype(bf)
    wt_stats[:, 2 : 2 + IN] = L.astype(bf)

    rank_rhs = np.zeros((P, OUT), bf)
    rank_rhs[0, :] = bpp.astype(bf)
    rank_rhs[1, :] = nb64.astype(bf)

    consts = np.zeros((P, 2), np.float32)
    consts[:, 0] = bbar
    consts[:, 1] = msq + EPS
    return wt_main, wt_stats, rank_rhs, consts


def pack_x(xs):
    """[rows, IN] bf16 -> [rows//2, 2*IN] pair-tile lhsT layout.

    For each 256-row block, partition p of the SBUF tile holds DRAM rows
    (2p, 2p+1); compute tile A = even rows, B = odd rows.  Columns 0:IN are
    A.T, columns IN:2*IN are B.T."""
    rows = xs.shape[0]
    g = rows // (2 * P)
    x4 = xs.reshape(g, P, 2, P)
    xp = np.empty((g, P, 2 * P), ml_dtypes.bfloat16)
    xp[:, :, :P] = x4[:, :, 0, :].transpose(0, 2, 1)
    xp[:, :, P:] = x4[:, :, 1, :].transpose(0, 2, 1)
    return xp.reshape(rows // 2, 2 * P)


def kernel(x, y, linear_w, linear_b, norm_w, norm_b):
    global LAST_RESULT
    from concourse.bass_utils import run_bass_kernel_spmd

    bf = ml_dtypes.bfloat16
    x = np.ascontiguousarray(x, np.float32).astype(bf)
    y = np.ascontiguousarray(y, np.float32).astype(bf)
    nb_rows = x.shape[0]
    assert nb_rows % N_CORES == 0
    bs = nb_rows // N_CORES

    wt_main, wt_stats, rank_rhs, consts = _host_prep(
        x, y, linear_w, linear_b, norm_w, norm_b
    )

    key = (bs, os.environ.get("KSTORE", "gp"))
    if key not in _CACHE:
        _CACHE[key] = _build_nc(bs)
    nc = _CACHE[key]

    in_maps = []
    for c in range(N_CORES):
        xs = x[c * bs : (c + 1) * bs]
        in_maps.append(
            {
                "xp": pack_x(xs),
                "y": y[c * bs : (c + 1) * bs],
                "wt_main": wt_main,
                "wt_stats": wt_stats,
                "rank_rhs": rank_rhs,
                "consts": consts,
            }
        )

    res = run_bass_kernel_spmd(nc, in_maps, core_ids=list(range(N_CORES)))
    LAST_RESULT = res
    out = np.concatenate([res.results[c]["out"] for c in range(N_CORES)], axis=0)
    return out
